# revision 16
# baseline (speedup 1.0000x reference)
"""TRN2 Bass kernel for the NonLocal (full N^2 attention) block.

Contract: kernel(**inputs) takes the FULL inputs (x:[4,128,64,64] plus 4x
(W:[128,128], b:[128])) and returns the full [4,128,64,64] output.

Sharding: 8 cores = 4 batches x 2 query-halves (2048 query rows each).
Each core receives the full x[b] (keys/values span all 4096 positions) and
its query slice; outputs are disjoint [128,2048] slices -> no collectives.

Per-core pipeline (v7):
  x and the conv weights arrive as f16 PAIRS packed into an f32 DRAM tensor.

  PHI CONV FOLDED: softmax over keys is invariant to per-query shifts, so
  s_kq = theta_q . phi_k == x_k . theta'_q (mod per-query consts) with
    theta' = A*(Wph^T Wth) x_q + A*(Wph^T bth)      (W4/b4, host-side)
  The sc matmul uses the resident x chunk directly as lhsT; phi never
  exists on chip.

  DENOMINATOR RIDES THE AV MATMUL: Wg2 = 0.5*Wo@Wg (output conv folded;
  normalization commutes through the channel conv) is SVD-truncated to
  rank 127: Wg2 ~= Ag @ Bg (indistinguishable at f16 precision,
  sigma_128/sigma_1 ~ 1e-4). gT holds [Bg x | ones], so the single
  accumulating AV matmul produces zt = [[Bg X E]; [sum_k E]] -- numerator
  rows AND the softmax denominator in f32 PSUM. The old bf16 sum-tree
  (~59 DVE adds) and ones-absorb matmuls are gone.

  theta' = W4 @ Xq + b4            [C, 2048]   f16 (A = Schraudolph scale)
  gT     = [X^T @ Bg^T | 1]        [4096, 128] bf16
  per 1024-wide q-group, streaming over 32 key-chunks of 128:
    scT  = x_chunk^T @ theta'_q    [128, 1024] (= A*score + per-q const)
    E    = exp(scT/A)              ACT op (scale=1/A), OR on flagged chunks
           bitcast_bf16(int16(scT + BEXP))     (Schraudolph exp on the DVE)
    zt  += gT_chunk^T @ E          [128, 1024] (PSUM accumulation)
  tail per qg: ztb = bf16(zt) (ACT); rr = 1/zt[127] (DVE row recip);
    wt = Ag @ ztb (PE), rbb = ones^T rr (PE f32r broadcast matmul),
    out = (tanh(wt*rbb + 0.5*bo_eff) + 1) * (Xq/2)

The chunk stream is software-pipelined: sc leads exp by 1 step and av by
EXP_AV_SKEW steps so the in-order PE queue rides through ACT queueing
jitter. A WARM_MMS dummy-matmul burst at t=0 keeps the PE busy through the
input DMA so the HAM clock gate opens once (1.2 -> 2.4 GHz).
"""

import sys

for _p in ("/opt/trn_rl_repo",):
    if _p not in sys.path:
        sys.path.insert(0, _p)

import numpy as np

import concourse.bass as bass
import concourse.bacc as bacc
import concourse.mybir as mybir
import concourse.tile as tile
from concourse.bass_utils import run_bass_kernel_spmd

F32 = mybir.dt.float32
F32R = mybir.dt.float32r
F16 = mybir.dt.float16
BF16 = mybir.dt.bfloat16
I16 = mybir.dt.int16
AF = mybir.ActivationFunctionType

B, C, H, W = 4, 128, 64, 64
N = H * W            # 4096 key/value positions
NQ = N // 2          # 2048 query rows per core
QG = 512             # PSUM bank / max fp32 moving dim
GW = 2 * QG          # q-group width (1024)
NQG = NQ // GW       # 2 q-groups
MC = 32              # key chunks of 128
N_CORES = 8

SC_DT = F16
AV_DT = BF16

# Schraudolph fake-exp constants (bf16): bits(e^x) ~= int16(A*x + BEXP).
# A is folded into theta' host-side; BEXP tuned for round-to-nearest convert.
A_EXP = 128.0 / np.log(2.0)          # 184.6650
BEXP_C0 = -5.6                       # centering offset, calibrated on host
BEXP = 16256.0 + BEXP_C0

# chunks whose exp runs on the DVE (Schraudolph) instead of the ACT engine.
# ACT sustains ~0.85 chunks/step; the rest go to the DVE (which has slack
# now that the sum tree is gone). qg1's last chunks stay on ACT so the DVE
# is free when zt closes.
_DVE_MCS = (1, 3, 6, 9, 12, 14, 17, 20, 22, 25, 28, 30)
DVE_EXP = frozenset([(0, mc) for mc in _DVE_MCS]
                    + [(1, mc) for mc in _DVE_MCS if mc <= 28])

WARM_MMS = 4         # dummy FD=512 matmuls at t=0 to warm the PE clock gate
                     # AND bridge the DMA-gated conv phase so HAM never
                     # re-throttles before the stream saturates
EXP_AV_SKEW = 4      # steps between a chunk's sc matmul and its av matmul

# packed input column offsets, in f32 columns. x and the conv weights are
# packed as f16 PAIRS host-side (halves the DMA, kills the on-chip casts);
# each core's x[b] is PERMUTED so its own query half comes first.
OFF_X16 = 0              # [C, N/2] f32 cols = [C, N] f16 x
OFF_WB = OFF_X16 + N // 2    # f16 Bg^T (col 127 zero-padded), 64 f32 cols
OFF_AG = OFF_WB + C // 2     # f16 Agpad^T (row 127 of Agpad == 0)
OFF_W4 = OFF_AG + C // 2     # f16 (A*Wth.T@Wph) = W4.T
OFF_B4 = OFF_W4 + C // 2     # f32 A*(Wph.T@bth)
OFF_BO = OFF_B4 + 1      # holds 0.5*bo_eff (for the tanh-based sigmoid)
NW = OFF_BO + 1          # 2242

_CACHE = {}


def build_program():
    nc = bacc.Bacc("TRN2", target_bir_lowering=False, debug=False,
                   num_devices=N_CORES)

    inp = nc.declare_dram_parameter("inp", [C, NW], F32, isOutput=False)
    out = nc.declare_dram_parameter("out", [C, NQ], F32, isOutput=True)

    with tile.TileContext(nc) as tc:
        with (
            tc.tile_pool(name="const", bufs=1) as const,
            tc.tile_pool(name="big", bufs=1) as big,
            tc.tile_pool(name="epool", bufs=6) as epool,
            tc.tile_pool(name="zpool", bufs=2) as zpool,
            tc.tile_pool(name="tailp", bufs=2) as tailp,
        ):
            inp_s = big.tile([C, NW], F32)
            # weights/biases first (tiny), then f16-x: two leading eighths
            # (so the first convs start earliest), then quarters.
            nc.sync.dma_start(out=inp_s[:, OFF_WB:], in_=inp[:, OFF_WB:])
            for sl in (slice(0, N // 16), slice(N // 16, N // 8),
                       slice(N // 8, N // 4), slice(N // 4, N // 2)):
                nc.sync.dma_start(out=inp_s[:, sl], in_=inp[:, sl])

            xf16 = inp_s[:, OFF_X16:OFF_X16 + N // 2].bitcast(F16)  # [C, N]
            xq16 = xf16[:, :NQ]
            wb16 = inp_s[:, OFF_WB:OFF_WB + C // 2].bitcast(F16)
            ag16 = inp_s[:, OFF_AG:OFF_AG + C // 2].bitcast(F16)
            w416 = inp_s[:, OFF_W4:OFF_W4 + C // 2].bitcast(F16)
            b4_s = inp_s[:, OFF_B4:OFF_B4 + 1]
            bo_s = inp_s[:, OFF_BO:OFF_BO + 1]

            # warm-up source + ones row for the f32r broadcast matmul
            warm_src = const.tile([C, QG], AV_DT)
            nc.vector.memset(warm_src, 0.0)
            ones1 = const.tile([1, C], F32)
            nc.vector.memset(ones1, 1.0)

            # preload the exp ACT table set while input DMA streams
            warm = const.tile([1, 1], F32)
            nc.scalar.activation(out=warm, in_=ones1[0:1, 0:1], func=AF.Exp)

            th_s = big.tile([C, NQ], SC_DT)
            # 0.5*xq (f16), for out = (tanh(...) + 1) * (xq/2)
            xqh_s = big.tile([C, NQ], F16)
            gT_s = big.tile([C, MC, C], AV_DT)
            # ones column (channel 0) of every gT chunk; written once,
            # the per-group casts only cover channels 1..127. (Row 0 is
            # used because engine APs need quadrant-aligned partition
            # bases -- zt[127:128] would be rejected by the verifier.)
            nc.vector.memset(gT_s[:, :, 0:1], 1.0)

            # ---- PSUM pools: sc 4 banks + zt 2 + mm 2 = 8 ----
            with (
                tc.tile_pool(name="ps_sc", bufs=2, space="PSUM") as ps_sc,
                tc.tile_pool(name="ps_y", bufs=1, space="PSUM") as ps_y,
                tc.tile_pool(name="ps_mm", bufs=2, space="PSUM") as ps_mm,
            ):
                # ---- PE warm-up burst: dummy matmuls on memset data keep the
                # PE busy so the HAM clock-gate opens during input DMA. ----
                warm_ps = ps_y.tile([C, GW], F32, name="warm", tag="yt")
                for wi in range(WARM_MMS):
                    nc.tensor.matmul(warm_ps[:, :QG], lhsT=warm_src[:, :C],
                                     rhs=warm_src, start=True, stop=True)

                # ---- convs, interleaved in DMA-arrival order. theta' slice
                # j needs x f32 cols [256j, 256j+256); gT group g needs
                # [256g, 256g+256). ----
                def emit_th_conv(j):
                    ps = ps_mm.tile([C, QG], F32, tag="mm", name=f"cvt_{j}")
                    nc.tensor.matmul(
                        ps, lhsT=w416, rhs=xf16[:, j * QG:(j + 1) * QG],
                        start=True, stop=True,
                    )
                    with nc.allow_low_precision(reason="theta storage dtype"):
                        nc.scalar.activation(
                            out=th_s[:, j * QG:(j + 1) * QG], in_=ps,
                            func=AF.Identity, bias=b4_s)

                conv_state = {"gt4": 0}

                def emit_gt_conv4():
                    g4 = conv_state["gt4"]
                    conv_state["gt4"] += 1
                    ps = ps_mm.tile([C, 4, C], F32, tag="mm", name=f"cvg_{g4}")
                    for k in range(4):
                        mc = 4 * g4 + k
                        nc.tensor.matmul(
                            ps[:, k, :],
                            lhsT=xf16[:, mc * C:(mc + 1) * C], rhs=wb16,
                            start=True, stop=True,
                        )
                    with nc.allow_low_precision(reason="gT storage dtype"):
                        nc.vector.tensor_copy(
                            out=gT_s[:, 4 * g4:4 * g4 + 4, 1:],
                            in_=ps[:, :, 1:])

                emit_th_conv(0)
                emit_gt_conv4()          # g0 (keys 0-511)
                emit_th_conv(1)
                emit_gt_conv4()          # g1
                emit_th_conv(2)
                emit_th_conv(3)

                def ensure_convs(mc):
                    # stay a few chunks ahead of the attention stream; finish
                    # early so the mm psum pool frees up for the tail tiles
                    while conv_state["gt4"] < min((mc + 8) // 4 + 2, MC // 4):
                        emit_gt_conv4()

                ensure_convs(-1)

                def emit_tail_a(st):
                    # zt rows 1..127 = Bg X E, row 0 = sum_k E (all f32).
                    qg = st["qg"]
                    ztb = zpool.tile([C, GW], BF16, name=f"ztb_{qg}",
                                     tag="ztb")
                    with nc.allow_low_precision(reason="zt bf16 staging"):
                        nc.scalar.activation(out=ztb, in_=st["zt"],
                                             func=AF.Identity)
                    rr = tailp.tile([1, GW], F32, name=f"rr_{qg}", tag="rr")
                    nc.vector.reciprocal_approx_fast(
                        out=rr, in_=st["zt"][0:1, :])
                    st["ztb"], st["rr"] = ztb, rr

                def emit_tail_b(st):
                    qg = st["qg"]
                    q0 = qg * GW
                    ztb, rr = st["ztb"], st["rr"]
                    for h in range(2):
                        sl = slice(h * QG, (h + 1) * QG)
                        # wt = Ag @ ztb[1:]  (row 0 of Agpad^T is 0)
                        wt = ps_mm.tile([C, QG], F32, tag="mm",
                                        name=f"wt_{qg}_{h}")
                        nc.tensor.matmul(wt, lhsT=ag16, rhs=ztb[:, sl],
                                         start=True, stop=True)
                        # rbb = broadcast of 1/r over partitions (K=1 fp32 mm;
                        # PE is idle at tail time, the 4-pass fp32 cost is fine)
                        rbb = ps_mm.tile([C, QG], F32, tag="mm",
                                         name=f"rbb_{qg}_{h}")
                        nc.tensor.matmul(rbb, lhsT=ones1,
                                         rhs=rr[:, sl],
                                         start=True, stop=True)
                        # DVE can read only one PSUM operand; stage rbb
                        # through SBUF on the (tail-idle) ACT engine.
                        rbc = tailp.tile([C, QG], F32, name=f"rbc_{qg}_{h}",
                                         tag="rbc")
                        nc.scalar.activation(out=rbc, in_=rbb,
                                             func=AF.Identity)
                        yn = tailp.tile([C, QG], F32, name=f"yn_{qg}_{h}",
                                        tag="yn")
                        nc.vector.tensor_mul(out=yn, in0=wt, in1=rbc)
                        # sigmoid(wy+bo)*xq == (tanh((wy+bo)/2) + 1) * (xq/2);
                        # the 0.5 factor is folded into Wg2, bo_s holds
                        # 0.5*bo_eff. tanh shares the exp ACT table set.
                        tn = tailp.tile([C, QG], F32, name=f"tn_{qg}_{h}",
                                        tag="tn")
                        nc.scalar.activation(
                            out=tn, in_=yn, func=AF.Tanh, bias=bo_s,
                            scale=1.0,
                        )
                        o = tailp.tile([C, QG], F32, name=f"o_{qg}_{h}",
                                       tag="o")
                        nc.vector.scalar_tensor_tensor(
                            out=o, in0=tn, scalar=1.0,
                            in1=xqh_s[:, q0 + h * QG:q0 + (h + 1) * QG],
                            op0=mybir.AluOpType.add, op1=mybir.AluOpType.mult,
                        )
                        eng = nc.sync if h % 2 == 0 else nc.gpsimd
                        eng.dma_start(
                            out=out[:, q0 + h * QG:q0 + (h + 1) * QG], in_=o)

                # ---- software-pipelined attention stream ----
                qstate = []
                for qg in range(NQG):
                    qstate.append({
                        "qg": qg,
                        "zt": ps_y.tile([C, GW], F32, name=f"zt_{qg}",
                                        tag="yt"),
                        "sc": {},
                        "et": {},
                    })

                def stage_sc(st, mc):
                    qg = st["qg"]
                    if qg == 0:
                        ensure_convs(mc)
                        if mc == 12:
                            # xqh is first needed by the qg0 tail; emitting it
                            # here keeps the DVE clear during startup
                            with nc.allow_low_precision(reason="xq/2 dtype"):
                                nc.vector.tensor_scalar_mul(
                                    out=xqh_s, in0=xq16, scalar1=0.5)
                    sc = ps_sc.tile([C, GW], F32, name=f"sc_{qg}_{mc}",
                                    tag="sc")
                    st["sc"][mc] = sc
                    q0 = qg * GW
                    for h in range(2):
                        nc.tensor.matmul(
                            sc[:, h * QG:(h + 1) * QG],
                            lhsT=xf16[:, mc * C:(mc + 1) * C],
                            rhs=th_s[:, q0 + h * QG:q0 + (h + 1) * QG],
                            start=True, stop=True,
                        )

                def stage_exp(st, mc):
                    qg = st["qg"]
                    sc = st["sc"].pop(mc)
                    et = epool.tile([C, GW], AV_DT, name=f"et_{qg}_{mc}",
                                    tag="et")
                    st["et"][mc] = et
                    with nc.allow_low_precision(reason="exp output dtype"):
                        if (qg, mc) in DVE_EXP:
                            # Schraudolph: bits(e^x) = int16(A*x + BEXP);
                            # max(.,0) clamps scores < -88 to +0.0 (the
                            # int16 would go negative -> NaN bit pattern)
                            nc.vector.tensor_scalar(
                                out=et.bitcast(I16), in0=sc, scalar1=BEXP,
                                scalar2=0.0, op0=mybir.AluOpType.add,
                                op1=mybir.AluOpType.max)
                        else:
                            nc.scalar.activation(out=et, in_=sc, func=AF.Exp,
                                                 scale=1.0 / A_EXP)

                def stage_post(st, mc):
                    qg = st["qg"]
                    if qg == 1 and mc == EXP_AV_SKEW:
                        emit_tail_a(qstate[0])
                    if qg == 1 and mc == EXP_AV_SKEW + 4:
                        emit_tail_b(qstate[0])
                    et = st["et"].pop(mc)
                    for h in range(2):
                        nc.tensor.matmul(
                            st["zt"][:, h * QG:(h + 1) * QG],
                            lhsT=gT_s[:, mc, :],
                            rhs=et[:, h * QG:(h + 1) * QG],
                            start=(mc == 0), stop=(mc == MC - 1),
                        )

                # skew: sc leads exp by 1 step and av by EXP_AV_SKEW, so the
                # in-order PE queue rides through ACT latency + queue jitter.
                steps = [(qg, mc) for qg in range(NQG) for mc in range(MC)]
                nsteps = len(steps)
                for i in range(nsteps + EXP_AV_SKEW):
                    if i < nsteps:
                        stage_sc(qstate[steps[i][0]], steps[i][1])
                    if 0 <= i - 1 < nsteps:
                        stage_exp(qstate[steps[i - 1][0]], steps[i - 1][1])
                    j = i - EXP_AV_SKEW
                    if 0 <= j < nsteps:
                        stage_post(qstate[steps[j][0]], steps[j][1])
                emit_tail_a(qstate[1])
                emit_tail_b(qstate[1])

    nc.compile()
    return nc


def get_program():
    if "nc" not in _CACHE:
        _CACHE["nc"] = build_program()
    return _CACHE["nc"]


def _f16_pack(a):
    # [C, k] f16 -> [C, k/2] f32 bit-view (little-endian pair packing
    # matches the device-side .bitcast(F16))
    a = np.ascontiguousarray(a.astype(np.float16))
    return a.view(np.float32)


def make_in_maps(x, Wg, bg, Wth, bth, Wph, bph, Wo, bo):
    xr = np.ascontiguousarray(x.reshape(B, C, N), np.float32)
    bo_eff = (Wo.astype(np.float64) @ bg.astype(np.float64)
              + bo.astype(np.float64)).astype(np.float32)
    Wg2 = 0.5 * (Wo.astype(np.float64) @ Wg.astype(np.float64))
    # rank-127 split Wg2 ~= Ag @ Bg frees one AV output row for the
    # softmax denominator (sigma_128/sigma_1 ~ 1e-4 -> negligible).
    U, S, Vt = np.linalg.svd(Wg2)
    Ag = U[:, :C - 1] * np.sqrt(S[:C - 1])          # [128, 127]
    Bg = np.sqrt(S[:C - 1])[:, None] * Vt[:C - 1]   # [127, 128]
    BgT = np.zeros((C, C))
    BgT[:, 1:] = Bg.T
    Agpad = np.zeros((C, C))
    Agpad[:, 1:] = Ag
    # phi fold: s = x_k . theta'_q (+ per-query const, dies in softmax)
    #   theta' = A*(Wph^T Wth) x + A*(Wph^T bth)
    W4T = A_EXP * (Wth.astype(np.float64).T @ Wph.astype(np.float64))
    b4 = A_EXP * (Wph.astype(np.float64).T @ bth.astype(np.float64))
    wblock = np.concatenate([
        _f16_pack(BgT),
        _f16_pack(Agpad.T),
        _f16_pack(W4T),
        b4.reshape(C, 1).astype(np.float32),
        (0.5 * bo_eff).reshape(C, 1),
    ], axis=1)
    in_maps = []
    for core in range(N_CORES):
        b, qh = divmod(core, 2)
        mine = xr[b][:, qh * NQ:(qh + 1) * NQ]
        other = xr[b][:, (1 - qh) * NQ:(2 - qh) * NQ]
        x16 = _f16_pack(np.concatenate([mine, other], axis=1))
        packed = np.concatenate([x16, wblock], axis=1)
        in_maps.append({"inp": np.ascontiguousarray(packed)})
    return in_maps


def run(trace=False, **inputs):
    nc = get_program()
    in_maps = make_in_maps(**inputs)
    res = run_bass_kernel_spmd(nc, in_maps, core_ids=list(range(N_CORES)),
                               trace=trace)
    full = np.empty((B, C, N), np.float32)
    for core in range(N_CORES):
        b, qh = divmod(core, 2)
        full[b][:, qh * NQ:(qh + 1) * NQ] = res.results[core]["out"]
    return full.reshape(B, C, H, W), res


def kernel(**inputs) -> np.ndarray:
    out, _ = run(trace=False, **inputs)
    return out


# revision 17
# speedup vs baseline: 1.0408x; 1.0408x over previous
"""TRN2 Bass kernel for the NonLocal (full N^2 attention) block.

Contract: kernel(**inputs) takes the FULL inputs (x:[4,128,64,64] plus 4x
(W:[128,128], b:[128])) and returns the full [4,128,64,64] output.

Sharding: 8 cores = 4 batches x 2 query-halves (2048 query rows each).
Each core receives the full x[b] (keys/values span all 4096 positions) and
its query slice; outputs are disjoint [128,2048] slices -> no collectives.

Per-core pipeline (v7):
  x and the conv weights arrive as f16 PAIRS packed into an f32 DRAM tensor.

  PHI CONV FOLDED: softmax over keys is invariant to per-query shifts, so
  s_kq = theta_q . phi_k == x_k . theta'_q (mod per-query consts) with
    theta' = A*(Wph^T Wth) x_q + A*(Wph^T bth)      (W4/b4, host-side)
  The sc matmul uses the resident x chunk directly as lhsT; phi never
  exists on chip.

  DENOMINATOR RIDES THE AV MATMUL: Wg2 = 0.5*Wo@Wg (output conv folded;
  normalization commutes through the channel conv) is SVD-truncated to
  rank 127: Wg2 ~= Ag @ Bg (indistinguishable at f16 precision,
  sigma_128/sigma_1 ~ 1e-4). gT holds [Bg x | ones], so the single
  accumulating AV matmul produces zt = [[Bg X E]; [sum_k E]] -- numerator
  rows AND the softmax denominator in f32 PSUM. The old bf16 sum-tree
  (~59 DVE adds) and ones-absorb matmuls are gone.

  theta' = W4 @ Xq + b4            [C, 2048]   f16 (A = Schraudolph scale)
  gT     = [X^T @ Bg^T | 1]        [4096, 128] bf16
  per 1024-wide q-group, streaming over 32 key-chunks of 128:
    scT  = x_chunk^T @ theta'_q    [128, 1024] (= A*score + per-q const)
    E    = exp(scT/A)              ACT op (scale=1/A), OR on flagged chunks
           bitcast_bf16(int16(scT + BEXP))     (Schraudolph exp on the DVE)
    zt  += gT_chunk^T @ E          [128, 1024] (PSUM accumulation)
  tail per qg: ztb = bf16(zt) (ACT); rr = 1/zt[127] (DVE row recip);
    wt = Ag @ ztb (PE), rbb = ones^T rr (PE f32r broadcast matmul),
    out = (tanh(wt*rbb + 0.5*bo_eff) + 1) * (Xq/2)

The chunk stream is software-pipelined: sc leads exp by 1 step and av by
EXP_AV_SKEW steps so the in-order PE queue rides through ACT queueing
jitter. A WARM_MMS dummy-matmul burst at t=0 keeps the PE busy through the
input DMA so the HAM clock gate opens once (1.2 -> 2.4 GHz).
"""

import sys

for _p in ("/opt/trn_rl_repo",):
    if _p not in sys.path:
        sys.path.insert(0, _p)

import numpy as np

import concourse.bass as bass
import concourse.bacc as bacc
import concourse.mybir as mybir
import concourse.tile as tile
from concourse.bass_utils import run_bass_kernel_spmd

F32 = mybir.dt.float32
F32R = mybir.dt.float32r
F16 = mybir.dt.float16
BF16 = mybir.dt.bfloat16
I16 = mybir.dt.int16
AF = mybir.ActivationFunctionType

B, C, H, W = 4, 128, 64, 64
N = H * W            # 4096 key/value positions
NQ = N // 2          # 2048 query rows per core
QG = 512             # PSUM bank / max fp32 moving dim
GW = 2 * QG          # q-group width (1024)
NQG = NQ // GW       # 2 q-groups
MC = 32              # key chunks of 128
N_CORES = 8

SC_DT = F16
AV_DT = BF16

# Schraudolph fake-exp constants (bf16): bits(e^x) ~= int16(A*x + BEXP).
# A is folded into theta' host-side; BEXP tuned for round-to-nearest convert.
A_EXP = 128.0 / np.log(2.0)          # 184.6650
BEXP_C0 = -5.6                       # centering offset, calibrated on host
BEXP = 16256.0 + BEXP_C0

# chunks whose exp runs on the DVE (Schraudolph) instead of the ACT engine.
# ACT sustains ~0.85 chunks/step; the rest go to the DVE (which has slack
# now that the sum tree is gone). qg1's last chunks stay on ACT so the DVE
# is free when zt closes.
DVE_EXP = frozenset(
    [(0, mc) for mc in (1, 3, 6, 9, 12, 15, 18, 21, 24, 27, 30)]
    + [(1, mc) for mc in (1, 3, 6, 9, 12, 15, 18, 21, 24, 26, 28, 30)]
)

WARM_MMS = 4         # dummy FD=512 matmuls at t=0 to warm the PE clock gate
                     # AND bridge the DMA-gated conv phase so HAM never
                     # re-throttles before the stream saturates
EXP_AV_SKEW = 4      # steps between a chunk's sc matmul and its av matmul

# packed input column offsets, in f32 columns. x and the conv weights are
# packed as f16 PAIRS host-side (halves the DMA, kills the on-chip casts);
# each core's x[b] is PERMUTED so its own query half comes first.
OFF_X16 = 0              # [C, N/2] f32 cols = [C, N] f16 x
OFF_WB = OFF_X16 + N // 2    # f16 Bg^T (col 127 zero-padded), 64 f32 cols
OFF_AG = OFF_WB + C // 2     # f16 Agpad^T (row 127 of Agpad == 0)
OFF_W4 = OFF_AG + C // 2     # f16 (A*Wth.T@Wph) = W4.T
OFF_B4 = OFF_W4 + C // 2     # f32 A*(Wph.T@bth)
OFF_BO = OFF_B4 + 1      # holds 0.5*bo_eff (for the tanh-based sigmoid)
NW = OFF_BO + 1          # 2242

_CACHE = {}


def build_program():
    nc = bacc.Bacc("TRN2", target_bir_lowering=False, debug=False,
                   num_devices=N_CORES)

    inp = nc.declare_dram_parameter("inp", [C, NW], F32, isOutput=False)
    out = nc.declare_dram_parameter("out", [C, NQ], F32, isOutput=True)

    with tile.TileContext(nc) as tc:
        with (
            tc.tile_pool(name="const", bufs=1) as const,
            tc.tile_pool(name="big", bufs=1) as big,
            tc.tile_pool(name="epool", bufs=6) as epool,
            tc.tile_pool(name="zpool", bufs=2) as zpool,
            tc.tile_pool(name="tailp", bufs=2) as tailp,
        ):
            inp_s = big.tile([C, NW], F32)
            # weights/biases first (tiny), then f16-x: two leading eighths
            # (so the first convs start earliest), then quarters.
            nc.sync.dma_start(out=inp_s[:, OFF_WB:], in_=inp[:, OFF_WB:])
            for sl in (slice(0, N // 16), slice(N // 16, N // 8),
                       slice(N // 8, N // 4), slice(N // 4, N // 2)):
                nc.sync.dma_start(out=inp_s[:, sl], in_=inp[:, sl])

            xf16 = inp_s[:, OFF_X16:OFF_X16 + N // 2].bitcast(F16)  # [C, N]
            xq16 = xf16[:, :NQ]
            wb16 = inp_s[:, OFF_WB:OFF_WB + C // 2].bitcast(F16)
            ag16 = inp_s[:, OFF_AG:OFF_AG + C // 2].bitcast(F16)
            w416 = inp_s[:, OFF_W4:OFF_W4 + C // 2].bitcast(F16)
            b4_s = inp_s[:, OFF_B4:OFF_B4 + 1]
            bo_s = inp_s[:, OFF_BO:OFF_BO + 1]

            # warm-up source + ones row for the f32r broadcast matmul
            warm_src = const.tile([C, QG], AV_DT)
            nc.vector.memset(warm_src, 0.0)
            ones1 = const.tile([1, C], BF16)
            nc.vector.memset(ones1, 1.0)

            # preload the exp ACT table set while input DMA streams
            warm = const.tile([1, 1], F32)
            nc.scalar.activation(out=warm, in_=warm_src[0:1, 0:1],
                                 func=AF.Exp)

            th_s = big.tile([C, NQ], SC_DT)
            # 0.5*xq (f16), for out = (tanh(...) + 1) * (xq/2)
            xqh_s = big.tile([C, NQ], F16)
            gT_s = big.tile([C, MC, C], AV_DT)
            # ones column (channel 0) of every gT chunk; written once,
            # the per-group casts only cover channels 1..127. (Row 0 is
            # used because engine APs need quadrant-aligned partition
            # bases -- zt[127:128] would be rejected by the verifier.)
            nc.vector.memset(gT_s[:, :, 0:1], 1.0)

            # ---- PSUM pools: sc 4 banks + zt 2 + mm 2 = 8 ----
            with (
                tc.tile_pool(name="ps_sc", bufs=2, space="PSUM") as ps_sc,
                tc.tile_pool(name="ps_y", bufs=1, space="PSUM") as ps_y,
                tc.tile_pool(name="ps_mm", bufs=2, space="PSUM") as ps_mm,
            ):
                # ---- PE warm-up burst: dummy matmuls on memset data keep the
                # PE busy so the HAM clock-gate opens during input DMA. ----
                warm_ps = ps_y.tile([C, GW], F32, name="warm", tag="yt")

                def warm_fill(k):
                    # filler matmuls burn otherwise-idle (DMA-gated) PE
                    # time so the HAM activity window stays busy
                    for wi in range(k):
                        nc.tensor.matmul(warm_ps[:, :QG],
                                         lhsT=warm_src[:, :C],
                                         rhs=warm_src, start=True,
                                         stop=True)

                warm_fill(WARM_MMS)

                # ---- convs, interleaved in DMA-arrival order. theta' slice
                # j needs x f32 cols [256j, 256j+256); gT group g needs
                # [256g, 256g+256). ----
                def emit_th_conv(j):
                    ps = ps_mm.tile([C, QG], F32, tag="mm", name=f"cvt_{j}")
                    nc.tensor.matmul(
                        ps, lhsT=w416, rhs=xf16[:, j * QG:(j + 1) * QG],
                        start=True, stop=True,
                    )
                    with nc.allow_low_precision(reason="theta storage dtype"):
                        nc.scalar.activation(
                            out=th_s[:, j * QG:(j + 1) * QG], in_=ps,
                            func=AF.Identity, bias=b4_s)

                conv_state = {"gt4": 0}

                def emit_gt_conv4():
                    g4 = conv_state["gt4"]
                    conv_state["gt4"] += 1
                    ps = ps_mm.tile([C, 4, C], F32, tag="mm", name=f"cvg_{g4}")
                    for k in range(4):
                        mc = 4 * g4 + k
                        nc.tensor.matmul(
                            ps[:, k, :],
                            lhsT=xf16[:, mc * C:(mc + 1) * C], rhs=wb16,
                            start=True, stop=True,
                        )
                    with nc.allow_low_precision(reason="gT storage dtype"):
                        nc.vector.tensor_copy(
                            out=gT_s[:, 4 * g4:4 * g4 + 4, 1:],
                            in_=ps[:, :, 1:])

                emit_th_conv(0)
                warm_fill(1)
                emit_gt_conv4()          # g0 (keys 0-511)
                warm_fill(1)
                emit_th_conv(1)
                emit_gt_conv4()          # g1
                warm_fill(1)
                emit_th_conv(2)
                emit_th_conv(3)

                def ensure_convs(mc):
                    # stay a few chunks ahead of the attention stream; finish
                    # early so the mm psum pool frees up for the tail tiles
                    while conv_state["gt4"] < min((mc + 8) // 4 + 2, MC // 4):
                        emit_gt_conv4()

                ensure_convs(-1)

                def emit_tail_a(st):
                    # zt rows 1..127 = Bg X E, row 0 = sum_k E (all f32).
                    qg = st["qg"]
                    ztb = zpool.tile([C, GW], BF16, name=f"ztb_{qg}",
                                     tag="ztb")
                    with nc.allow_low_precision(reason="zt bf16 staging"):
                        nc.scalar.activation(out=ztb, in_=st["zt"],
                                             func=AF.Identity)
                    # r row staged to bf16 SBUF for the broadcast matmul
                    rr = tailp.tile([1, GW], BF16, name=f"rr_{qg}", tag="rr")
                    with nc.allow_low_precision(reason="r bf16 staging"):
                        nc.vector.tensor_copy(out=rr, in_=st["zt"][0:1, :])
                    st["ztb"], st["rr"] = ztb, rr

                def emit_tail_b(st):
                    qg = st["qg"]
                    q0 = qg * GW
                    ztb, rr = st["ztb"], st["rr"]
                    for h in range(2):
                        sl = slice(h * QG, (h + 1) * QG)
                        # wt = Ag @ ztb[1:]  (row 0 of Agpad^T is 0)
                        wt = ps_mm.tile([C, QG], F32, tag="mm",
                                        name=f"wt_{qg}_{h}")
                        nc.tensor.matmul(wt, lhsT=ag16, rhs=ztb[:, sl],
                                         start=True, stop=True)
                        # rB = broadcast of r over partitions (K=1 bf16 mm),
                        # then 1/rB on the DVE straight into SBUF so the
                        # yn multiply has only one PSUM operand.
                        rbb = ps_mm.tile([C, QG], F32, tag="mm",
                                         name=f"rbb_{qg}_{h}")
                        nc.tensor.matmul(rbb, lhsT=ones1,
                                         rhs=rr[:, sl],
                                         start=True, stop=True)
                        rbc = tailp.tile([C, QG], F32, name=f"rbc_{qg}_{h}",
                                         tag="rbc")
                        nc.vector.reciprocal_approx_fast(out=rbc, in_=rbb)
                        yn = tailp.tile([C, QG], F32, name=f"yn_{qg}_{h}",
                                        tag="yn")
                        nc.vector.tensor_mul(out=yn, in0=wt, in1=rbc)
                        # sigmoid(wy+bo)*xq == (tanh((wy+bo)/2) + 1) * (xq/2);
                        # the 0.5 factor is folded into Wg2, bo_s holds
                        # 0.5*bo_eff. tanh shares the exp ACT table set.
                        tn = tailp.tile([C, QG], F32, name=f"tn_{qg}_{h}",
                                        tag="tn")
                        nc.scalar.activation(
                            out=tn, in_=yn, func=AF.Tanh, bias=bo_s,
                            scale=1.0,
                        )
                        o = tailp.tile([C, QG], F32, name=f"o_{qg}_{h}",
                                       tag="o")
                        nc.vector.scalar_tensor_tensor(
                            out=o, in0=tn, scalar=1.0,
                            in1=xqh_s[:, q0 + h * QG:q0 + (h + 1) * QG],
                            op0=mybir.AluOpType.add, op1=mybir.AluOpType.mult,
                        )
                        eng = nc.sync if h % 2 == 0 else nc.gpsimd
                        eng.dma_start(
                            out=out[:, q0 + h * QG:q0 + (h + 1) * QG], in_=o)

                # ---- software-pipelined attention stream ----
                qstate = []
                for qg in range(NQG):
                    qstate.append({
                        "qg": qg,
                        "zt": ps_y.tile([C, GW], F32, name=f"zt_{qg}",
                                        tag="yt"),
                        "sc": {},
                        "et": {},
                    })

                def stage_sc(st, mc):
                    qg = st["qg"]
                    if qg == 0:
                        ensure_convs(mc)
                        if mc == 12:
                            # xqh is first needed by the qg0 tail; emitting it
                            # here keeps the DVE clear during startup
                            with nc.allow_low_precision(reason="xq/2 dtype"):
                                nc.vector.tensor_scalar_mul(
                                    out=xqh_s, in0=xq16, scalar1=0.5)
                    sc = ps_sc.tile([C, GW], F32, name=f"sc_{qg}_{mc}",
                                    tag="sc")
                    st["sc"][mc] = sc
                    q0 = qg * GW
                    for h in range(2):
                        nc.tensor.matmul(
                            sc[:, h * QG:(h + 1) * QG],
                            lhsT=xf16[:, mc * C:(mc + 1) * C],
                            rhs=th_s[:, q0 + h * QG:q0 + (h + 1) * QG],
                            start=True, stop=True,
                        )

                def stage_exp(st, mc):
                    qg = st["qg"]
                    sc = st["sc"].pop(mc)
                    et = epool.tile([C, GW], AV_DT, name=f"et_{qg}_{mc}",
                                    tag="et")
                    st["et"][mc] = et
                    with nc.allow_low_precision(reason="exp output dtype"):
                        if (qg, mc) in DVE_EXP:
                            # Schraudolph: bits(e^x) = int16(A*x + BEXP);
                            # max(.,0) clamps scores < -88 to +0.0 (the
                            # int16 would go negative -> NaN bit pattern)
                            nc.vector.tensor_scalar(
                                out=et.bitcast(I16), in0=sc, scalar1=BEXP,
                                scalar2=0.0, op0=mybir.AluOpType.add,
                                op1=mybir.AluOpType.max)
                        else:
                            nc.scalar.activation(out=et, in_=sc, func=AF.Exp,
                                                 scale=1.0 / A_EXP)

                def stage_post(st, mc):
                    qg = st["qg"]
                    if qg == 1 and mc == EXP_AV_SKEW:
                        emit_tail_a(qstate[0])
                    if qg == 1 and mc == EXP_AV_SKEW + 4:
                        emit_tail_b(qstate[0])
                    et = st["et"].pop(mc)
                    for h in range(2):
                        nc.tensor.matmul(
                            st["zt"][:, h * QG:(h + 1) * QG],
                            lhsT=gT_s[:, mc, :],
                            rhs=et[:, h * QG:(h + 1) * QG],
                            start=(mc == 0), stop=(mc == MC - 1),
                        )

                # skew: sc leads exp by 1 step and av by EXP_AV_SKEW, so the
                # in-order PE queue rides through ACT latency + queue jitter.
                steps = [(qg, mc) for qg in range(NQG) for mc in range(MC)]
                nsteps = len(steps)
                for i in range(nsteps + EXP_AV_SKEW):
                    if i < nsteps:
                        stage_sc(qstate[steps[i][0]], steps[i][1])
                    if 0 <= i - 1 < nsteps:
                        stage_exp(qstate[steps[i - 1][0]], steps[i - 1][1])
                    j = i - EXP_AV_SKEW
                    if 0 <= j < nsteps:
                        stage_post(qstate[steps[j][0]], steps[j][1])
                emit_tail_a(qstate[1])
                emit_tail_b(qstate[1])

    nc.compile()
    return nc


def get_program():
    if "nc" not in _CACHE:
        _CACHE["nc"] = build_program()
    return _CACHE["nc"]


def _f16_pack(a):
    # [C, k] f16 -> [C, k/2] f32 bit-view (little-endian pair packing
    # matches the device-side .bitcast(F16))
    a = np.ascontiguousarray(a.astype(np.float16))
    return a.view(np.float32)


def make_in_maps(x, Wg, bg, Wth, bth, Wph, bph, Wo, bo):
    xr = np.ascontiguousarray(x.reshape(B, C, N), np.float32)
    bo_eff = (Wo.astype(np.float64) @ bg.astype(np.float64)
              + bo.astype(np.float64)).astype(np.float32)
    Wg2 = 0.5 * (Wo.astype(np.float64) @ Wg.astype(np.float64))
    # rank-127 split Wg2 ~= Ag @ Bg frees one AV output row for the
    # softmax denominator (sigma_128/sigma_1 ~ 1e-4 -> negligible).
    U, S, Vt = np.linalg.svd(Wg2)
    Ag = U[:, :C - 1] * np.sqrt(S[:C - 1])          # [128, 127]
    Bg = np.sqrt(S[:C - 1])[:, None] * Vt[:C - 1]   # [127, 128]
    BgT = np.zeros((C, C))
    BgT[:, 1:] = Bg.T
    Agpad = np.zeros((C, C))
    Agpad[:, 1:] = Ag
    # phi fold: s = x_k . theta'_q (+ per-query const, dies in softmax)
    #   theta' = A*(Wph^T Wth) x + A*(Wph^T bth)
    W4T = A_EXP * (Wth.astype(np.float64).T @ Wph.astype(np.float64))
    b4 = A_EXP * (Wph.astype(np.float64).T @ bth.astype(np.float64))
    wblock = np.concatenate([
        _f16_pack(BgT),
        _f16_pack(Agpad.T),
        _f16_pack(W4T),
        b4.reshape(C, 1).astype(np.float32),
        (0.5 * bo_eff).reshape(C, 1),
    ], axis=1)
    in_maps = []
    for core in range(N_CORES):
        b, qh = divmod(core, 2)
        mine = xr[b][:, qh * NQ:(qh + 1) * NQ]
        other = xr[b][:, (1 - qh) * NQ:(2 - qh) * NQ]
        x16 = _f16_pack(np.concatenate([mine, other], axis=1))
        packed = np.concatenate([x16, wblock], axis=1)
        in_maps.append({"inp": np.ascontiguousarray(packed)})
    return in_maps


def run(trace=False, **inputs):
    nc = get_program()
    in_maps = make_in_maps(**inputs)
    res = run_bass_kernel_spmd(nc, in_maps, core_ids=list(range(N_CORES)),
                               trace=trace)
    full = np.empty((B, C, N), np.float32)
    for core in range(N_CORES):
        b, qh = divmod(core, 2)
        full[b][:, qh * NQ:(qh + 1) * NQ] = res.results[core]["out"]
    return full.reshape(B, C, H, W), res


def kernel(**inputs) -> np.ndarray:
    out, _ = run(trace=False, **inputs)
    return out
